# revision 1
# baseline (speedup 1.0000x reference)
"""DeltaGradientDescentMemory Trainium2 kernel.

Math: per step t (T=8192, dk=dv=1024, H=8 heads):
    kn_t = k_t / max(||k_t||, eps)
    r_t^h = W_h^T kn_t           (read before update)
    W_h  += kn_t (beta v_t - alpha r_t^h)^T
    out_t = mean_h r_t^h

The recurrence is linear in W0 and the dynamics are head-independent, so
mean_h r^h equals the single recurrence started from S0 = mean_h W0_h.
The dv dimension is fully independent given kn, so the 8 cores shard dv
(128 columns each) with zero cross-core communication.

On-device chunked delta rule (chunk C=128, state Shat = alpha*S in PSUM fp32):
    A  = K K^T                  (Gram, fp8e4 DoubleRow on PE, 2x rate)
    PT = (ALPHA/KS^2) * triu(A, 1)  ( = (alpha L)^T, bf16 )
    Gh = K Shat                 ( = alpha * K S, bf16 )
    X0 = V - Gh
    U  = X0 - PT.T@X0           (1-term Neumann solve of (I+alpha L)U = X0)
    R  = (Gh + PT.T@U) / alpha  -> output rows (bf16)
    Shat += (alpha K)^T U       (PSUM accumulate, bf16)

The ktt stream holds KS*kn (KS=16, an exact bf16 exponent shift) so the
on-chip fp8e4 Gram copy is a plain DVE cast with good e4m3 range; the
KS/KS^2 factors are folded into the snapshot-refresh scale (1/KS) and
the triu-mask / cross-term scales (ALPHA/KS^2).

Engine budget per chunk (64 chunks, ~1.64us DMA-bound ceiling):
    PE:  gram fp8 DR + G + state + solve/cross/R     ~1.2us
    ACT: sbA+sbB refresh (scale 1/KS) + rout         ~1.5us
    DVE: x0 + u + pt-mask + fp8 cast slice           ~1.5us
    Pool: ct cross-scale + r DMA dispatch            ~0.4us
"""
import contextlib

import numpy as np
import ml_dtypes
import orjson

import concourse.bass as bass
import concourse.mybir as mybir
import concourse.tile as tile
from concourse.bass_utils import run_bass_kernel_spmd
from concourse.masks import make_identity, make_upper_triangular

ALPHA = 0.1
EPS = 1e-12
T, DK, DV, H = 8192, 1024, 1024, 8
C = 128                  # chunk length
NCH = T // C             # 64 chunks
NOCT = NCH // 8          # kT tiles hold 8 chunks each
DVS = DV // 8            # dv shard per core
NJ = DK // 128           # 8 dk tiles
KS = 16.0                # fp8 kn scale
GRAM_SCALE = ALPHA / (KS * KS)

BF16 = mybir.dt.bfloat16
FP32 = mybir.dt.float32
FP8 = mybir.dt.float8e4

# ---------------------------------------------------------------------------
# Walrus in this container accepts at most ONE sync-wait per instruction, but
# Tile emits several. Legalize the serialized BIR: hoist all but the last wait
# of an instruction onto fresh single-wait EventSemaphore instructions placed
# just before it (same engine stream, order preserved).
_mw_counter = [0]


def _legalize_multiwait_json(bir_bytes: bytes) -> bytes:
    j = orjson.loads(bir_bytes)
    changed = False
    for fn in j.get("functions", []):
        for bb in fn.get("blocks", []):
            out = []
            for inst in bb.get("instructions", []):
                si = inst.get("sync_info")
                waits = si.get("on_wait") if si else None
                if waits and len(waits) > 1:
                    changed = True
                    for w in waits[:-1]:
                        _mw_counter[0] += 1
                        out.append({
                            "debug": inst.get("debug", 0),
                            "engine": inst["engine"],
                            "ins": [],
                            "outs": [],
                            "name": f"mwsplit-{_mw_counter[0]}",
                            "opcode": "EventSemaphore",
                            "sync_info": {"on_update": [], "on_wait": [w]},
                        })
                    si["on_wait"] = waits[-1:]
                out.append(inst)
            bb["instructions"] = out
    return orjson.dumps(j) if changed else bir_bytes


_orig_to_json_bytes = bass.Bass.to_json_bytes


def _patched_to_json_bytes(self):
    return _legalize_multiwait_json(_orig_to_json_bytes(self))


bass.Bass.to_json_bytes = _patched_to_json_bytes


# ---------------------------------------------------------------------------
def build_kernel(n_chunks: int = NCH, reps: int = 1, zero_init: bool = True,
                 use_cross: bool = True, fp8_gram: bool = True,
                 fast_chain: bool = True, hw_reps: int = 1) -> bass.Bass:
    """hw_reps > 1 wraps the whole per-pass body in a tc.For_i hardware
    loop (constant NEFF size), for steady-state timing measurements."""
    nc = bass.Bass(trn_type="TRN2")
    ktt = nc.dram_tensor("ktt", [NOCT, NJ, 128, 8 * C], BF16, kind="ExternalInput")
    ktl = nc.dram_tensor("ktl", [NCH, C, DK], BF16, kind="ExternalInput")
    # v and r live in DRAM as [C, NCH, DVS] so multi-chunk loads/stores are
    # one contiguous >=1KB run per partition (<=512B runs pay 2x DMA latency).
    vb = nc.dram_tensor("vb", [C, NCH, DVS], BF16, kind="ExternalInput")
    s0 = nc.dram_tensor("s0", [NJ, 128, DVS], FP32, kind="ExternalInput")
    r = nc.dram_tensor("r", [C, NCH, DVS], BF16, kind="ExternalOutput")

    with tile.TileContext(nc) as tc:
        with (
            tc.tile_pool(name="const", bufs=1) as cpool,
            tc.tile_pool(name="kt", bufs=3) as ktpool,
            tc.tile_pool(name="ktf8", bufs=3) as kf8pool,
            tc.tile_pool(name="ktl", bufs=3) as ktlpool,
            tc.tile_pool(name="vb", bufs=3) as vbpool,
            tc.tile_pool(name="sb", bufs=1) as sbpool,
            tc.tile_pool(name="wk", bufs=2) as wk,
            tc.tile_pool(name="routq", bufs=2) as routpool,
            tc.tile_pool(name="psS", bufs=1, space="PSUM") as psSp,
            tc.tile_pool(name="psG", bufs=2, space="PSUM") as psGp,
            tc.tile_pool(name="psA", bufs=1, space="PSUM") as psAp,
            tc.tile_pool(name="psX", bufs=1, space="PSUM") as psXp,
        ):
            # constants
            idn = cpool.tile([128, 128], FP32, tag="idn", name="idn")
            make_identity(nc, idn[:])
            idnb = cpool.tile([128, 128], BF16, tag="idnb", name="idnb")
            make_identity(nc, idnb[:])
            # ktt carries KS*kn, so the Gram PSUM is KS^2*A for both the
            # fp8 and bf16 gram paths; GRAM_SCALE folds that back out in
            # the ACT copy of psA; mu01 then masks PT to strict-upper.
            mu01 = cpool.tile([128, 128], BF16, tag="mu01", name="mu01")
            make_upper_triangular(nc, mu01[:], val=1.0, diag=False)

            # persistent state tiles (PSUM accumulator + bf16 snapshots)
            HALF = NJ // 2 * DVS  # 512
            psS0 = psSp.tile([128, HALF], FP32, tag="psS0", name="psS0")
            psS1 = psSp.tile([128, HALF], FP32, tag="psS1", name="psS1")
            psS_half = [psS0, psS1]
            s0_sb = cpool.tile([128, NJ * DVS], FP32, tag="s0", name="s0sb")
            if zero_init:
                nc.gpsimd.memset(s0_sb[:], 0.0)
            else:
                nc.sync.dma_start(
                    s0_sb[:].rearrange("p (j c) -> p j c", j=NJ),
                    s0[:].rearrange("j p c -> p j c"),
                )

            sbA = sbpool.tile([128, HALF], BF16, tag="sbA", name="sbA")
            sbB = sbpool.tile([128, HALF], BF16, tag="sbB", name="sbB")

            def SB(j):
                t = sbA if j < 4 else sbB
                return t[:, (j % 4) * DVS:(j % 4 + 1) * DVS]

            n_total = n_chunks * reps

            def chunk_qh(ci):
                c = ci % n_chunks
                return divmod(c, 8)

            # per-chunk tile handles, filled as the pipeline advances
            kt_tiles = {}    # octo -> sbuf tile (bf16 KS*kn, [dk, t])
            kf8_tiles = {}   # octo -> sbuf tile (fp8 KS*kn, [dk, t])
            ktl_tiles = {}   # octo -> sbuf tile (alpha*kn rows)
            vb_tiles = {}    # group (16 chunks) -> sbuf tile
            rout_tiles = {}  # group (8 chunks) -> bf16 out staging
            pt_tiles = {}    # ci -> PT bf16
            ct_tiles = {}    # ci -> scaled cross-gram bf16
            psG_tiles = {}   # ci -> psum with G-base (+cross +R later)
            u_tiles = {}     # ci -> U bf16
            y_tiles = {}     # ci -> psU accumulator (pre-solve in PSUM)
            ctn_tiles = {}   # ci -> negated scaled cross-gram bf16
            if fast_chain:
                assert use_cross, "fast_chain requires use_cross"

            def KT(ci, j):
                _, h = chunk_qh(ci)
                ktv = kt_tiles[ci // 8][:].rearrange("p (j c) -> p j c", j=NJ)
                return ktv[:, j, h * C:(h + 1) * C]

            def K8(ci, jp, w):
                """fp8 [128, 2, w*C] slice: dk-subtiles (2jp, 2jp+1), chunk
                window of w chunks starting at ci's slot in its octo tile."""
                _, h = chunk_qh(ci)
                k8v = kf8_tiles[ci // 8][:].rearrange("p (j c) -> p j c", j=NJ)
                return k8v[:, 2 * jp:2 * jp + 2, h * C:(h + w) * C]

            def conv_chunk(t):
                """DVE cast of chunk t's kt slice to fp8."""
                if not fp8_gram or t >= n_total:
                    return
                oc, s = t // 8, chunk_qh(t)[1]
                if oc not in kt_tiles:
                    return
                if oc not in kf8_tiles:
                    kf8_tiles[oc] = kf8pool.tile(
                        [128, NJ * 8 * C], FP8, tag="kf8", name="ktf8_sb")
                kv = kt_tiles[oc][:].rearrange("p (j c) -> p j c", j=NJ)
                fv = kf8_tiles[oc][:].rearrange("p (j c) -> p j c", j=NJ)
                nc.gpsimd.tensor_copy(
                    fv[:, :, s * C:(s + 1) * C], kv[:, :, s * C:(s + 1) * C])

            def load_oct(oc):
                if oc * 8 >= n_total or oc in kt_tiles:
                    return
                q = chunk_qh(oc * 8)[0]
                t = ktpool.tile([128, NJ * 8 * C], BF16, tag="kt", name="ktt_sb")
                nc.sync.dma_start(
                    t[:].rearrange("p (j c) -> p j c", j=NJ),
                    ktt[q].rearrange("j p c -> p j c"),
                )
                kt_tiles[oc] = t

            def load_ktl(oc):
                if oc * 8 >= n_total or oc in ktl_tiles:
                    return
                c0 = (oc * 8) % n_chunks
                t = ktlpool.tile([C, 8 * DK], BF16, tag="ktl", name="ktl_sb")
                nc.sync.dma_start(
                    t[:].rearrange("p (i d) -> p i d", i=8),
                    ktl[c0:c0 + 8].rearrange("i p d -> p i d"),
                )
                ktl_tiles[oc] = t

            def load_vb(grp):
                if grp * 16 >= n_total or grp in vb_tiles:
                    return
                c0 = (grp * 16) % n_chunks
                t = vbpool.tile([C, 16 * DVS], BF16, tag="vb", name="vb_sb")
                nc.sync.dma_start(
                    t[:].rearrange("p (i d) -> p i d", i=16),
                    vb[:, c0:c0 + 16, :],
                )
                vb_tiles[grp] = t

            def dma_in(ci):
                """Issue prefetch DMAs one octo ahead of chunk ci."""
                if ci >= n_total:
                    return
                if ci % 8 == 0:
                    load_oct(ci // 8 + 1)
                    load_vb(ci // 16 + (1 if ci % 16 else 0))
                if ci % 8 == 4:
                    load_ktl(ci // 8 + 1)

            def gram(ci, psA):
                """A(ci) [+ cross A(ci, ci+1)] into psA."""
                q, h = chunk_qh(ci)
                have_next = ci + 1 < n_total
                if fp8_gram:
                    if have_next and use_cross and h < 7:
                        for jp in range(NJ // 2):
                            nc.tensor.matmul(
                                psA[:], K8(ci, jp, 1), K8(ci, jp, 2),
                                start=(jp == 0), stop=(jp == NJ // 2 - 1),
                                perf_mode=mybir.MatmulPerfMode.DoubleRow,
                            )
                    else:
                        for jp in range(NJ // 2):
                            nc.tensor.matmul(
                                psA[:, 0:C], K8(ci, jp, 1), K8(ci, jp, 1),
                                start=(jp == 0), stop=(jp == NJ // 2 - 1),
                                perf_mode=mybir.MatmulPerfMode.DoubleRow,
                                skip_group_check=True,
                            )
                        if have_next and use_cross:
                            for jp in range(NJ // 2):
                                nc.tensor.matmul(
                                    psA[:, C:2 * C], K8(ci, jp, 1),
                                    K8(ci + 1, jp, 1),
                                    start=(jp == 0), stop=(jp == NJ // 2 - 1),
                                    perf_mode=mybir.MatmulPerfMode.DoubleRow,
                                    skip_group_check=True,
                                )
                else:
                    if have_next and use_cross and h < 7:
                        ktv = kt_tiles[ci // 8][:].rearrange("p (j c) -> p j c", j=NJ)
                        for j in range(NJ):
                            nc.tensor.matmul(
                                psA[:], KT(ci, j), ktv[:, j, h * C:(h + 2) * C],
                                start=(j == 0), stop=(j == NJ - 1),
                            )
                    else:
                        for j in range(NJ):
                            nc.tensor.matmul(
                                psA[:, 0:C], KT(ci, j), KT(ci, j),
                                start=(j == 0), stop=(j == NJ - 1),
                                skip_group_check=True,
                            )
                        if have_next and use_cross:
                            for j in range(NJ):
                                nc.tensor.matmul(
                                    psA[:, C:2 * C], KT(ci, j), KT(ci + 1, j),
                                    start=(j == 0), stop=(j == NJ - 1),
                                    skip_group_check=True,
                                )

            def prep_chunk(ci):
                """Everything for chunk ci that doesn't need u_{ci-1}:
                Gram(ci), PT/IMU/CT extraction, state refresh, G-base(ci),
                and the off-chain pre-solve psY = (I-PT)^T (V - G_base)."""
                if ci >= n_total:
                    return
                q, h = chunk_qh(ci)
                have_next = ci + 1 < n_total
                psA = psAp.tile([C, 2 * C], FP32, name="psA_t")
                gram(ci, psA)
                # ACT: scaled copies of the Gram PSUM -> bf16. The right
                # half of ac IS the scaled cross-gram CT for chunk ci+1;
                # acn is its negation (the psU accumulator subtracts it).
                aw = 2 * C if (have_next and use_cross) else C
                ac = wk.tile([C, 2 * C], BF16, tag="ac", name="ac_t")
                nc.scalar.activation(
                    ac[:, 0:aw], psA[:, 0:aw],
                    mybir.ActivationFunctionType.Copy, scale=GRAM_SCALE,
                )
                if have_next and use_cross:
                    ct_tiles[ci + 1] = ac[:, C:2 * C]
                    if fast_chain:
                        acn = wk.tile([C, C], BF16, tag="acn", name="acn_t")
                        nc.scalar.activation(
                            acn[:], psA[:, C:2 * C],
                            mybir.ActivationFunctionType.Copy,
                            scale=-GRAM_SCALE,
                        )
                        ctn_tiles[ci + 1] = acn
                # DVE (SBUF-only, 2x): PT = masked copy; IMU = I - PT.
                pt = wk.tile([C, C], BF16, tag="pt", name="pt_t")
                nc.vector.tensor_mul(pt[:], ac[:, 0:C], mu01[:])
                pt_tiles[ci] = pt
                if fast_chain:
                    imu = wk.tile([C, C], BF16, tag="imu", name="imu_t")
                    nc.vector.tensor_sub(imu[:], idnb[:], pt[:])
                # refresh state snapshot (S after chunk ci-2): halves on
                # ACT and DVE concurrently. scale 1/KS un-does the
                # host-side KS prescale of ktt.
                nc.scalar.activation(
                    sbA[:], psS0[:], mybir.ActivationFunctionType.Copy,
                    scale=1.0 / KS,
                )
                nc.vector.tensor_scalar_mul(sbB[:], psS1[:], 1.0 / KS)
                psG = psGp.tile([C, DVS], FP32, name="psG_t")
                for j in range(NJ):
                    nc.tensor.matmul(
                        psG[:], KT(ci, j), SB(j),
                        start=(j == 0), stop=False, skip_group_check=True,
                    )
                psG_tiles[ci] = psG
                if fast_chain:
                    # off-chain pre-solve into the u accumulator:
                    # psU = (I-PT)^T (V - G_base); the forward chain later
                    # adds -CT^T u_prev (via the negated cross-gram) and
                    # copies psU out (drops the 2nd-order PT^T CT^T term).
                    vb_t = vb_tiles[ci // 16][
                        :, (ci % 16) * DVS:(ci % 16 + 1) * DVS]
                    xt = wk.tile([C, DVS], BF16, tag="x0", name="xt_t")
                    nc.vector.tensor_sub(xt[:], vb_t, psG[:])
                    psU = psXp.tile([C, DVS], FP32, tag="psy", name="psU_t")
                    nc.tensor.matmul(psU[:], imu[:], xt[:],
                                     start=True, stop=(ci == 0),
                                     skip_group_check=True)
                    y_tiles[ci] = psU

            loop_ctx = (tc.For_i(0, hw_reps) if hw_reps > 1
                        else contextlib.nullcontext())
            with loop_ctx:
                # state init: psS = S0 (identity matmul, fp32 exact).
                # Re-runs per hw_reps iteration, resetting the accumulator.
                for j in range(NJ):
                    js = slice((j % 4) * DVS, (j % 4 + 1) * DVS)
                    nc.tensor.matmul(
                        psS_half[j // 4][:, js], idn[:],
                        s0_sb[:, j * DVS:(j + 1) * DVS],
                        start=True, stop=False, skip_group_check=True,
                    )

                # prologue: inputs + fp8 casts + prep for chunk 0.
                # Order = chunk 0's needs first (oct0, vb, ktl), then
                # the first prefetch; this sets the steady-state lag.
                load_oct(0)
                load_vb(0)
                load_ktl(0)
                load_oct(1)
                conv_chunk(0)
                conv_chunk(1)
                conv_chunk(2)
                prep_chunk(0)

                for ci in range(n_total):
                    q, h = chunk_qh(ci)
                    dma_in(ci + 1)

                    psG = psG_tiles.pop(ci)
                    pt = pt_tiles.pop(ci)
                    if fast_chain:
                        # forward chain: psU -= CT^T u_prev (negated-ct
                        # matmul accumulate), then u = copy(psU). The
                        # R-path cross lands in psG off the chain.
                        psU = y_tiles.pop(ci)
                        u = wk.tile([C, DVS], BF16, tag="u", name="u_t")
                        if ci > 0:
                            ct = ct_tiles.pop(ci)
                            ctn = ctn_tiles.pop(ci)
                            u_prev = u_tiles.pop(ci - 1)
                            nc.tensor.matmul(psU[:], ctn[:], u_prev[:],
                                             start=False, stop=True,
                                             skip_group_check=True)
                            nc.vector.tensor_copy(u[:], psU[:])
                            nc.tensor.matmul(
                                psG[:], ct, u_prev[:],
                                start=False, stop=False, skip_group_check=True,
                            )
                        else:
                            nc.vector.tensor_copy(u[:], psU[:])
                        u_tiles[ci] = u
                        prep_chunk(ci + 1)
                    else:
                        # cross-term: psG += alpha*K_ci K_{ci-1}^T @ U_{ci-1}
                        if ci > 0 and use_cross:
                            nc.tensor.matmul(
                                psG[:], ct_tiles.pop(ci), u_tiles.pop(ci - 1),
                                start=False, stop=False, skip_group_check=True,
                            )
                        if not use_cross:
                            u_tiles.pop(ci - 1, None)

                        # solve: X0 = V - G ; U = X0 - PT.T@X0
                        vb_t = vb_tiles[ci // 16][
                            :, (ci % 16) * DVS:(ci % 16 + 1) * DVS]
                        x0 = wk.tile([C, DVS], BF16, tag="x0", name="x0_t")
                        nc.vector.tensor_sub(x0[:], vb_t, psG[:])
                        psX0 = psXp.tile([C, DVS], FP32, tag="psx", name="psX_t")
                        nc.tensor.matmul(psX0[:], pt[:], x0[:], start=True, stop=True)
                        u = wk.tile([C, DVS], BF16, tag="u", name="u_t")
                        nc.vector.tensor_sub(u[:], x0[:], psX0[:])
                        u_tiles[ci] = u

                        # cross variant: prep ci+1 now - its refresh must READ
                        # psS (state after ci-1) BEFORE this chunk's update.
                        if use_cross:
                            prep_chunk(ci + 1)

                    # state update
                    ktl_t = ktl_tiles[ci // 8][:, h * DK:(h + 1) * DK]
                    for j in range(NJ):
                        js = slice((j % 4) * DVS, (j % 4 + 1) * DVS)
                        nc.tensor.matmul(
                            psS_half[j // 4][:, js], ktl_t[:, j * 128:(j + 1) * 128], u[:],
                            start=False, stop=(ci == n_total - 1), skip_group_check=True,
                        )

                    # outputs: R = G(+cross) + PT.T @ U
                    nc.tensor.matmul(
                        psG[:], pt[:], u[:],
                        start=False, stop=True, skip_group_check=True,
                    )
                    pos = ci % 8
                    if pos == 0:
                        rout_tiles[ci // 8] = routpool.tile(
                            [C, 8 * DVS], BF16, tag="routq", name="rout_q"
                        )
                    rout_q = rout_tiles[ci // 8]
                    nc.scalar.activation(
                        rout_q[:, pos * DVS:(pos + 1) * DVS], psG[:],
                        mybir.ActivationFunctionType.Copy, scale=1.0 / ALPHA,
                    )
                    if pos == 7:
                        c_base = (ci % n_chunks) - 7
                        nc.gpsimd.dma_start(
                            r[:, c_base:c_base + 8, :],
                            rout_tiles.pop(ci // 8)[:].rearrange(
                                "p (i d) -> p i d", i=8),
                        )

                    if not use_cross:
                        prep_chunk(ci + 1)

                    # fp8 cast at the tail of the iteration: a cast waiting on
                    # its kt DMA then only delays the NEXT iteration's DVE work.
                    # (+3: gram(ci+2), called at iteration ci+1, reads the fused
                    # fp8 window covering chunks ci+2 AND ci+3.)
                    conv_chunk(ci + 3)

                    # release consumed input tiles
                    if h == 7:
                        kt_tiles.pop(ci // 8 - 1, None)
                        kf8_tiles.pop(ci // 8 - 1, None)
                        ktl_tiles.pop(ci // 8 - 1, None)
                    if ci % 16 == 15:
                        vb_tiles.pop(ci // 16 - 1, None)

    return nc


_nc_cache = {}


def _get_nc(zero_init: bool):
    if zero_init not in _nc_cache:
        _nc_cache[zero_init] = build_kernel(zero_init=zero_init)
    return _nc_cache[zero_init]


def _prep_inputs(k: np.ndarray, v: np.ndarray, W0: np.ndarray):
    k = np.asarray(k, np.float32)
    v = np.asarray(v, np.float32)
    W0 = np.asarray(W0, np.float32)
    kn = k / np.maximum(np.linalg.norm(k, axis=-1, keepdims=True), EPS)
    knb = kn.astype(ml_dtypes.bfloat16)
    # kT tiles: [NOCT, NJ, 128, 8C] from KS*knb.T [DK, T]. The KS factor
    # is an exact bf16 exponent shift; the kernel folds 1/KS back in.
    ktts = (KS * knb.astype(np.float32)).astype(ml_dtypes.bfloat16)
    ktt = np.ascontiguousarray(
        ktts.T.reshape(NJ, 128, NOCT, 8 * C).transpose(2, 0, 1, 3)
    )
    ktl = (ALPHA * knb.astype(np.float32)).astype(ml_dtypes.bfloat16) \
        .reshape(NCH, C, DK)
    s0_full = (ALPHA * W0.mean(axis=0)).astype(np.float32)  # [DK, DV]
    shared = {"ktt": ktt, "ktl": np.ascontiguousarray(ktl)}
    per_core = []
    for i in range(8):
        cs = slice(i * DVS, (i + 1) * DVS)
        # [C, NCH, DVS]: per-partition contiguous multi-chunk runs
        vbi = v[:, cs].astype(ml_dtypes.bfloat16) \
            .reshape(NCH, C, DVS).transpose(1, 0, 2)
        s0i = np.ascontiguousarray(s0_full[:, cs]).reshape(NJ, 128, DVS)
        per_core.append(
            {**shared, "vb": np.ascontiguousarray(vbi), "s0": s0i})
    return per_core


def run(k, v, W0, trace=False, **kwargs):
    nc = _get_nc(zero_init=not np.any(np.asarray(W0)))
    in_maps = _prep_inputs(k, v, W0)
    res = run_bass_kernel_spmd(nc, in_maps, core_ids=list(range(8)),
                               trace=trace, **kwargs)
    out = np.concatenate(
        [res.results[i]["r"].astype(np.float32)     # [C, NCH, DVS]
         .transpose(1, 0, 2).reshape(T, DVS)
         for i in range(8)], axis=1
    )
    return out, res


def kernel(k, v, W0):
    out, _ = run(k, v, W0)
    return out.astype(np.float32)



# revision 9
# speedup vs baseline: 1.1268x; 1.1268x over previous
"""DeltaGradientDescentMemory Trainium2 kernel.

Math: per step t (T=8192, dk=dv=1024, H=8 heads):
    kn_t = k_t / max(||k_t||, eps)
    r_t^h = W_h^T kn_t           (read before update)
    W_h  += kn_t (beta v_t - alpha r_t^h)^T
    out_t = mean_h r_t^h

The recurrence is linear in W0 and the dynamics are head-independent, so
mean_h r^h equals the single recurrence started from S0 = mean_h W0_h.
The dv dimension is fully independent given kn, so the 8 cores shard dv
(128 columns each) with zero cross-core communication.

On-device chunked delta rule (chunk C=128, state Shat = alpha*S in PSUM fp32):
    A  = K K^T                  (Gram, fp8e4 DoubleRow on PE, 2x rate)
    PT = (ALPHA/KS^2) * triu(A, 1)  ( = (alpha L)^T, bf16 )
    Gh = K Shat                 ( = alpha * K S, bf16 )
    X0 = V - Gh
    U  = X0 - PT.T@X0           (1-term Neumann solve of (I+alpha L)U = X0)
    R  = (Gh + PT.T@U) / alpha  -> output rows (bf16)
    Shat += (alpha K)^T U       (PSUM accumulate, bf16)

The ktt stream holds KS*kn (KS=16, an exact bf16 exponent shift) so the
on-chip fp8e4 Gram copy is a plain DVE cast with good e4m3 range; the
KS/KS^2 factors are folded into the snapshot-refresh scale (1/KS) and
the triu-mask / cross-term scales (ALPHA/KS^2).

Engine budget per chunk (64 chunks, ~1.64us DMA-bound ceiling):
    PE:  gram fp8 DR + G + state + solve/cross/R     ~1.2us
    ACT: sbA+sbB refresh (scale 1/KS) + rout         ~1.5us
    DVE: x0 + u + pt-mask + fp8 cast slice           ~1.5us
    Pool: ct cross-scale + r DMA dispatch            ~0.4us
"""
import contextlib

import numpy as np
import ml_dtypes
import orjson

import concourse.bass as bass
import concourse.mybir as mybir
import concourse.tile as tile
from concourse.bass_utils import run_bass_kernel_spmd
from concourse.masks import make_identity, make_upper_triangular

ALPHA = 0.1
EPS = 1e-12
T, DK, DV, H = 8192, 1024, 1024, 8
C = 128                  # chunk length
NCH = T // C             # 64 chunks
NOCT = NCH // 8          # kT tiles hold 8 chunks each
DVS = DV // 8            # dv shard per core
NJ = DK // 128           # 8 dk tiles
KS = 16.0                # fp8 kn scale
GRAM_SCALE = ALPHA / (KS * KS)

BF16 = mybir.dt.bfloat16
FP32 = mybir.dt.float32
FP8 = mybir.dt.float8e4

# ---------------------------------------------------------------------------
# Walrus in this container accepts at most ONE sync-wait per instruction, but
# Tile emits several. Legalize the serialized BIR: hoist all but the last wait
# of an instruction onto fresh single-wait EventSemaphore instructions placed
# just before it (same engine stream, order preserved).
_mw_counter = [0]


def _legalize_multiwait_json(bir_bytes: bytes) -> bytes:
    j = orjson.loads(bir_bytes)
    changed = False
    for fn in j.get("functions", []):
        for bb in fn.get("blocks", []):
            out = []
            for inst in bb.get("instructions", []):
                si = inst.get("sync_info")
                waits = si.get("on_wait") if si else None
                if waits and len(waits) > 1:
                    changed = True
                    for w in waits[:-1]:
                        _mw_counter[0] += 1
                        out.append({
                            "debug": inst.get("debug", 0),
                            "engine": inst["engine"],
                            "ins": [],
                            "outs": [],
                            "name": f"mwsplit-{_mw_counter[0]}",
                            "opcode": "EventSemaphore",
                            "sync_info": {"on_update": [], "on_wait": [w]},
                        })
                    si["on_wait"] = waits[-1:]
                out.append(inst)
            bb["instructions"] = out
    return orjson.dumps(j) if changed else bir_bytes


_orig_to_json_bytes = bass.Bass.to_json_bytes


def _patched_to_json_bytes(self):
    return _legalize_multiwait_json(_orig_to_json_bytes(self))


bass.Bass.to_json_bytes = _patched_to_json_bytes


# ---------------------------------------------------------------------------
def build_kernel(n_chunks: int = NCH, reps: int = 1, zero_init: bool = True,
                 use_cross: bool = True, fp8_gram: bool = False,
                 fast_chain: bool = True, hw_reps: int = 1,
                 r_dma_sp: bool = False, kt_bufs: int = 3) -> bass.Bass:
    # fp8_gram=False by default: the per-chunk GPSIMD fp8 casts were the
    # dominant HW serialization (332us/pass with, 127us/pass without —
    # measured via interleaved reps=8/16 slope). bf16 gram costs ~2x the
    # PE gram cycles but PE has slack.
    """hw_reps > 1 wraps the whole per-pass body in a tc.For_i hardware
    loop (constant NEFF size), for steady-state timing measurements."""
    nc = bass.Bass(trn_type="TRN2")
    # Fat DMA layouts: one contiguous 16KB run per partition per load
    # (2KB runs measured 328 GB/s; 16KB runs 416 GB/s on the same bytes).
    ktt = nc.dram_tensor("ktt", [NOCT, 128, NJ * 8 * C], BF16, kind="ExternalInput")
    ktl = nc.dram_tensor("ktl", [NOCT, C, 8 * DK], BF16, kind="ExternalInput")
    # v and r live in DRAM as [C, NCH, DVS] so multi-chunk loads/stores are
    # one contiguous >=1KB run per partition (<=512B runs pay 2x DMA latency).
    vb = nc.dram_tensor("vb", [C, NCH, DVS], BF16, kind="ExternalInput")
    s0 = nc.dram_tensor("s0", [NJ, 128, DVS], FP32, kind="ExternalInput")
    r = nc.dram_tensor("r", [C, NCH, DVS], BF16, kind="ExternalOutput")

    with tile.TileContext(nc) as tc:
        with (
            tc.tile_pool(name="const", bufs=1) as cpool,
            tc.tile_pool(name="kt", bufs=kt_bufs) as ktpool,
            tc.tile_pool(name="ktf8", bufs=3) as kf8pool,
            tc.tile_pool(name="ktl", bufs=kt_bufs) as ktlpool,
            tc.tile_pool(name="vb", bufs=3) as vbpool,
            tc.tile_pool(name="sb", bufs=1) as sbpool,
            tc.tile_pool(name="wk", bufs=2) as wk,
            tc.tile_pool(name="routq", bufs=2) as routpool,
            tc.tile_pool(name="psS", bufs=1, space="PSUM") as psSp,
            tc.tile_pool(name="psG", bufs=2, space="PSUM") as psGp,
            tc.tile_pool(name="psA", bufs=1, space="PSUM") as psAp,
            tc.tile_pool(name="psX", bufs=1, space="PSUM") as psXp,
        ):
            # constants
            idn = cpool.tile([128, 128], FP32, tag="idn", name="idn")
            make_identity(nc, idn[:])
            idnb = cpool.tile([128, 128], BF16, tag="idnb", name="idnb")
            make_identity(nc, idnb[:])
            # ktt carries KS*kn, so the Gram PSUM is KS^2*A for both the
            # fp8 and bf16 gram paths; GRAM_SCALE folds that back out in
            # the ACT copy of psA; mu01 then masks PT to strict-upper.
            mu01 = cpool.tile([128, 128], BF16, tag="mu01", name="mu01")
            make_upper_triangular(nc, mu01[:], val=1.0, diag=False)

            # persistent state tiles (PSUM accumulator + bf16 snapshots)
            HALF = NJ // 2 * DVS  # 512
            psS0 = psSp.tile([128, HALF], FP32, tag="psS0", name="psS0")
            psS1 = psSp.tile([128, HALF], FP32, tag="psS1", name="psS1")
            psS_half = [psS0, psS1]
            s0_sb = cpool.tile([128, NJ * DVS], FP32, tag="s0", name="s0sb")
            if zero_init:
                nc.gpsimd.memset(s0_sb[:], 0.0)
            else:
                nc.sync.dma_start(
                    s0_sb[:].rearrange("p (j c) -> p j c", j=NJ),
                    s0[:].rearrange("j p c -> p j c"),
                )

            sbA = sbpool.tile([128, HALF], BF16, tag="sbA", name="sbA")
            sbB = sbpool.tile([128, HALF], BF16, tag="sbB", name="sbB")

            def SB(j):
                t = sbA if j < 4 else sbB
                return t[:, (j % 4) * DVS:(j % 4 + 1) * DVS]

            n_total = n_chunks * reps

            def chunk_qh(ci):
                c = ci % n_chunks
                return divmod(c, 8)

            # per-chunk tile handles, filled as the pipeline advances
            kt_tiles = {}    # octo -> sbuf tile (bf16 KS*kn, [dk, t])
            kf8_tiles = {}   # octo -> sbuf tile (fp8 KS*kn, [dk, t])
            ktl_tiles = {}   # octo -> sbuf tile (alpha*kn rows)
            vb_tiles = {}    # group (16 chunks) -> sbuf tile
            rout_tiles = {}  # group (8 chunks) -> bf16 out staging
            pt_tiles = {}    # ci -> PT bf16
            ct_tiles = {}    # ci -> scaled cross-gram bf16
            psG_tiles = {}   # ci -> psum with G-base (+cross +R later)
            u_tiles = {}     # ci -> U bf16
            y_tiles = {}     # ci -> psU accumulator (pre-solve in PSUM)
            ctn_tiles = {}   # ci -> negated scaled cross-gram bf16
            if fast_chain:
                assert use_cross, "fast_chain requires use_cross"

            def KT(ci, j):
                _, h = chunk_qh(ci)
                ktv = kt_tiles[ci // 8][:].rearrange("p (j c) -> p j c", j=NJ)
                return ktv[:, j, h * C:(h + 1) * C]

            def K8(ci, jp, w):
                """fp8 [128, 2, w*C] slice: dk-subtiles (2jp, 2jp+1), chunk
                window of w chunks starting at ci's slot in its octo tile."""
                _, h = chunk_qh(ci)
                k8v = kf8_tiles[ci // 8][:].rearrange("p (j c) -> p j c", j=NJ)
                return k8v[:, 2 * jp:2 * jp + 2, h * C:(h + w) * C]

            def conv_chunk(t):
                """DVE cast of chunk t's kt slice to fp8."""
                if not fp8_gram or t >= n_total:
                    return
                oc, s = t // 8, chunk_qh(t)[1]
                if oc not in kt_tiles:
                    return
                if oc not in kf8_tiles:
                    kf8_tiles[oc] = kf8pool.tile(
                        [128, NJ * 8 * C], FP8, tag="kf8", name="ktf8_sb")
                kv = kt_tiles[oc][:].rearrange("p (j c) -> p j c", j=NJ)
                fv = kf8_tiles[oc][:].rearrange("p (j c) -> p j c", j=NJ)
                nc.gpsimd.tensor_copy(
                    fv[:, :, s * C:(s + 1) * C], kv[:, :, s * C:(s + 1) * C])

            def load_oct(oc):
                if oc * 8 >= n_total or oc in kt_tiles:
                    return
                q = chunk_qh(oc * 8)[0]
                t = ktpool.tile([128, NJ * 8 * C], BF16, tag="kt", name="ktt_sb")
                nc.sync.dma_start(t[:], ktt[q])
                kt_tiles[oc] = t

            def load_ktl(oc):
                if oc * 8 >= n_total or oc in ktl_tiles:
                    return
                q = chunk_qh(oc * 8)[0]
                t = ktlpool.tile([C, 8 * DK], BF16, tag="ktl", name="ktl_sb")
                nc.sync.dma_start(t[:], ktl[q])
                ktl_tiles[oc] = t

            def load_vb(grp):
                if grp * 16 >= n_total or grp in vb_tiles:
                    return
                c0 = (grp * 16) % n_chunks
                t = vbpool.tile([C, 16 * DVS], BF16, tag="vb", name="vb_sb")
                nc.sync.dma_start(
                    t[:].rearrange("p (i d) -> p i d", i=16),
                    vb[:, c0:c0 + 16, :],
                )
                vb_tiles[grp] = t

            def dma_in(ci):
                """Issue prefetch DMAs one octo ahead of chunk ci."""
                if ci >= n_total:
                    return
                if ci % 8 == 0:
                    load_oct(ci // 8 + 1)
                    load_vb(ci // 16 + (1 if ci % 16 else 0))
                if ci % 8 == 4:
                    load_ktl(ci // 8 + 1)

            def gram(ci, psA):
                """A(ci) [+ cross A(ci, ci+1)] into psA."""
                q, h = chunk_qh(ci)
                have_next = ci + 1 < n_total
                if fp8_gram:
                    if have_next and use_cross and h < 7:
                        for jp in range(NJ // 2):
                            nc.tensor.matmul(
                                psA[:], K8(ci, jp, 1), K8(ci, jp, 2),
                                start=(jp == 0), stop=(jp == NJ // 2 - 1),
                                perf_mode=mybir.MatmulPerfMode.DoubleRow,
                            )
                    else:
                        for jp in range(NJ // 2):
                            nc.tensor.matmul(
                                psA[:, 0:C], K8(ci, jp, 1), K8(ci, jp, 1),
                                start=(jp == 0), stop=(jp == NJ // 2 - 1),
                                perf_mode=mybir.MatmulPerfMode.DoubleRow,
                                skip_group_check=True,
                            )
                        if have_next and use_cross:
                            for jp in range(NJ // 2):
                                nc.tensor.matmul(
                                    psA[:, C:2 * C], K8(ci, jp, 1),
                                    K8(ci + 1, jp, 1),
                                    start=(jp == 0), stop=(jp == NJ // 2 - 1),
                                    perf_mode=mybir.MatmulPerfMode.DoubleRow,
                                    skip_group_check=True,
                                )
                else:
                    if have_next and use_cross and h < 7:
                        ktv = kt_tiles[ci // 8][:].rearrange("p (j c) -> p j c", j=NJ)
                        for j in range(NJ):
                            nc.tensor.matmul(
                                psA[:], KT(ci, j), ktv[:, j, h * C:(h + 2) * C],
                                start=(j == 0), stop=(j == NJ - 1),
                            )
                    else:
                        for j in range(NJ):
                            nc.tensor.matmul(
                                psA[:, 0:C], KT(ci, j), KT(ci, j),
                                start=(j == 0), stop=(j == NJ - 1),
                                skip_group_check=True,
                            )
                        if have_next and use_cross:
                            for j in range(NJ):
                                nc.tensor.matmul(
                                    psA[:, C:2 * C], KT(ci, j), KT(ci + 1, j),
                                    start=(j == 0), stop=(j == NJ - 1),
                                    skip_group_check=True,
                                )

            def prep_chunk(ci):
                """Everything for chunk ci that doesn't need u_{ci-1}:
                Gram(ci), PT/IMU/CT extraction, state refresh, G-base(ci),
                and the off-chain pre-solve psY = (I-PT)^T (V - G_base)."""
                if ci >= n_total:
                    return
                q, h = chunk_qh(ci)
                have_next = ci + 1 < n_total
                psA = psAp.tile([C, 2 * C], FP32, name="psA_t")
                gram(ci, psA)
                # ACT: scaled copies of the Gram PSUM -> bf16. The right
                # half of ac IS the scaled cross-gram CT for chunk ci+1;
                # acn is its negation (the psU accumulator subtracts it).
                aw = 2 * C if (have_next and use_cross) else C
                ac = wk.tile([C, 2 * C], BF16, tag="ac", name="ac_t")
                nc.scalar.activation(
                    ac[:, 0:aw], psA[:, 0:aw],
                    mybir.ActivationFunctionType.Copy, scale=GRAM_SCALE,
                )
                if have_next and use_cross:
                    ct_tiles[ci + 1] = ac[:, C:2 * C]
                    if fast_chain:
                        acn = wk.tile([C, C], BF16, tag="acn", name="acn_t")
                        nc.scalar.activation(
                            acn[:], psA[:, C:2 * C],
                            mybir.ActivationFunctionType.Copy,
                            scale=-GRAM_SCALE,
                        )
                        ctn_tiles[ci + 1] = acn
                # DVE (SBUF-only, 2x): PT = masked copy; IMU = I - PT.
                pt = wk.tile([C, C], BF16, tag="pt", name="pt_t")
                nc.vector.tensor_mul(pt[:], ac[:, 0:C], mu01[:])
                pt_tiles[ci] = pt
                if fast_chain:
                    imu = wk.tile([C, C], BF16, tag="imu", name="imu_t")
                    nc.vector.tensor_sub(imu[:], idnb[:], pt[:])
                # refresh state snapshot (S after chunk ci-2): halves on
                # ACT and DVE concurrently. scale 1/KS un-does the
                # host-side KS prescale of ktt.
                nc.scalar.activation(
                    sbA[:], psS0[:], mybir.ActivationFunctionType.Copy,
                    scale=1.0 / KS,
                )
                nc.vector.tensor_scalar_mul(sbB[:], psS1[:], 1.0 / KS)
                psG = psGp.tile([C, DVS], FP32, name="psG_t")
                for j in range(NJ):
                    nc.tensor.matmul(
                        psG[:], KT(ci, j), SB(j),
                        start=(j == 0), stop=False, skip_group_check=True,
                    )
                psG_tiles[ci] = psG
                if fast_chain:
                    # off-chain pre-solve into the u accumulator:
                    # psU = (I-PT)^T (V - G_base); the forward chain later
                    # adds -CT^T u_prev (via the negated cross-gram) and
                    # copies psU out (drops the 2nd-order PT^T CT^T term).
                    vb_t = vb_tiles[ci // 16][
                        :, (ci % 16) * DVS:(ci % 16 + 1) * DVS]
                    xt = wk.tile([C, DVS], BF16, tag="x0", name="xt_t")
                    nc.vector.tensor_sub(xt[:], vb_t, psG[:])
                    psU = psXp.tile([C, DVS], FP32, tag="psy", name="psU_t")
                    nc.tensor.matmul(psU[:], imu[:], xt[:],
                                     start=True, stop=(ci == 0),
                                     skip_group_check=True)
                    y_tiles[ci] = psU

            loop_ctx = (tc.For_i(0, hw_reps) if hw_reps > 1
                        else contextlib.nullcontext())
            with loop_ctx:
                # state init: psS = S0 (identity matmul, fp32 exact).
                # Re-runs per hw_reps iteration, resetting the accumulator.
                for j in range(NJ):
                    js = slice((j % 4) * DVS, (j % 4 + 1) * DVS)
                    nc.tensor.matmul(
                        psS_half[j // 4][:, js], idn[:],
                        s0_sb[:, j * DVS:(j + 1) * DVS],
                        start=True, stop=False, skip_group_check=True,
                    )

                # prologue: inputs + fp8 casts + prep for chunk 0.
                # Order = chunk 0's needs first (oct0, vb, ktl), then
                # the first prefetch; this sets the steady-state lag.
                load_oct(0)
                load_vb(0)
                load_ktl(0)
                load_oct(1)
                conv_chunk(0)
                conv_chunk(1)
                conv_chunk(2)
                prep_chunk(0)

                for ci in range(n_total):
                    q, h = chunk_qh(ci)
                    dma_in(ci + 1)

                    psG = psG_tiles.pop(ci)
                    pt = pt_tiles.pop(ci)
                    if fast_chain:
                        # forward chain: psU -= CT^T u_prev (negated-ct
                        # matmul accumulate), then u = copy(psU). The
                        # R-path cross lands in psG off the chain.
                        psU = y_tiles.pop(ci)
                        u = wk.tile([C, DVS], BF16, tag="u", name="u_t")
                        if ci > 0:
                            ct = ct_tiles.pop(ci)
                            ctn = ctn_tiles.pop(ci)
                            u_prev = u_tiles.pop(ci - 1)
                            nc.tensor.matmul(psU[:], ctn[:], u_prev[:],
                                             start=False, stop=True,
                                             skip_group_check=True)
                            nc.vector.tensor_copy(u[:], psU[:])
                            nc.tensor.matmul(
                                psG[:], ct, u_prev[:],
                                start=False, stop=False, skip_group_check=True,
                            )
                        else:
                            nc.vector.tensor_copy(u[:], psU[:])
                        u_tiles[ci] = u
                        prep_chunk(ci + 1)
                    else:
                        # cross-term: psG += alpha*K_ci K_{ci-1}^T @ U_{ci-1}
                        if ci > 0 and use_cross:
                            nc.tensor.matmul(
                                psG[:], ct_tiles.pop(ci), u_tiles.pop(ci - 1),
                                start=False, stop=False, skip_group_check=True,
                            )
                        if not use_cross:
                            u_tiles.pop(ci - 1, None)

                        # solve: X0 = V - G ; U = X0 - PT.T@X0
                        vb_t = vb_tiles[ci // 16][
                            :, (ci % 16) * DVS:(ci % 16 + 1) * DVS]
                        x0 = wk.tile([C, DVS], BF16, tag="x0", name="x0_t")
                        nc.vector.tensor_sub(x0[:], vb_t, psG[:])
                        psX0 = psXp.tile([C, DVS], FP32, tag="psx", name="psX_t")
                        nc.tensor.matmul(psX0[:], pt[:], x0[:], start=True, stop=True)
                        u = wk.tile([C, DVS], BF16, tag="u", name="u_t")
                        nc.vector.tensor_sub(u[:], x0[:], psX0[:])
                        u_tiles[ci] = u

                        # cross variant: prep ci+1 now - its refresh must READ
                        # psS (state after ci-1) BEFORE this chunk's update.
                        if use_cross:
                            prep_chunk(ci + 1)

                    # state update
                    ktl_t = ktl_tiles[ci // 8][:, h * DK:(h + 1) * DK]
                    for j in range(NJ):
                        js = slice((j % 4) * DVS, (j % 4 + 1) * DVS)
                        nc.tensor.matmul(
                            psS_half[j // 4][:, js], ktl_t[:, j * 128:(j + 1) * 128], u[:],
                            start=False, stop=(ci == n_total - 1), skip_group_check=True,
                        )

                    # outputs: R = G(+cross) + PT.T @ U
                    nc.tensor.matmul(
                        psG[:], pt[:], u[:],
                        start=False, stop=True, skip_group_check=True,
                    )
                    pos = ci % 8
                    if pos == 0:
                        rout_tiles[ci // 8] = routpool.tile(
                            [C, 8 * DVS], BF16, tag="routq", name="rout_q"
                        )
                    rout_q = rout_tiles[ci // 8]
                    nc.scalar.activation(
                        rout_q[:, pos * DVS:(pos + 1) * DVS], psG[:],
                        mybir.ActivationFunctionType.Copy, scale=1.0 / ALPHA,
                    )
                    if pos == 7:
                        c_base = (ci % n_chunks) - 7
                        r_eng = nc.sync if r_dma_sp else nc.gpsimd
                        r_eng.dma_start(
                            r[:, c_base:c_base + 8, :],
                            rout_tiles.pop(ci // 8)[:].rearrange(
                                "p (i d) -> p i d", i=8),
                        )

                    if not use_cross:
                        prep_chunk(ci + 1)

                    # fp8 cast at the tail of the iteration: a cast waiting on
                    # its kt DMA then only delays the NEXT iteration's DVE work.
                    # (+3: gram(ci+2), called at iteration ci+1, reads the fused
                    # fp8 window covering chunks ci+2 AND ci+3.)
                    conv_chunk(ci + 3)

                    # release consumed input tiles
                    if h == 7:
                        kt_tiles.pop(ci // 8 - 1, None)
                        kf8_tiles.pop(ci // 8 - 1, None)
                        ktl_tiles.pop(ci // 8 - 1, None)
                    if ci % 16 == 15:
                        vb_tiles.pop(ci // 16 - 1, None)

    return nc


_nc_cache = {}


def _get_nc(zero_init: bool):
    if zero_init not in _nc_cache:
        _nc_cache[zero_init] = build_kernel(zero_init=zero_init)
    return _nc_cache[zero_init]


def _prep_inputs(k: np.ndarray, v: np.ndarray, W0: np.ndarray):
    k = np.asarray(k, np.float32)
    v = np.asarray(v, np.float32)
    W0 = np.asarray(W0, np.float32)
    kn = k / np.maximum(np.linalg.norm(k, axis=-1, keepdims=True), EPS)
    knb = kn.astype(ml_dtypes.bfloat16)
    # kT tiles: [NOCT, NJ, 128, 8C] from KS*knb.T [DK, T]. The KS factor
    # is an exact bf16 exponent shift; the kernel folds 1/KS back in.
    ktts = (KS * knb.astype(np.float32)).astype(ml_dtypes.bfloat16)
    # fat layout: [NOCT, 128, NJ*8C] — per-partition 16KB contiguous run
    ktt = np.ascontiguousarray(
        ktts.T.reshape(NJ, 128, NOCT, 8 * C).transpose(2, 1, 0, 3)
        .reshape(NOCT, 128, NJ * 8 * C)
    )
    # fat layout: [NOCT, C, 8*DK] — per-partition 16KB contiguous run
    ktl = (ALPHA * knb.astype(np.float32)).astype(ml_dtypes.bfloat16) \
        .reshape(NOCT, 8, C, DK).transpose(0, 2, 1, 3) \
        .reshape(NOCT, C, 8 * DK)
    s0_full = (ALPHA * W0.mean(axis=0)).astype(np.float32)  # [DK, DV]
    shared = {"ktt": ktt, "ktl": np.ascontiguousarray(ktl)}
    per_core = []
    for i in range(8):
        cs = slice(i * DVS, (i + 1) * DVS)
        # [C, NCH, DVS]: per-partition contiguous multi-chunk runs
        vbi = v[:, cs].astype(ml_dtypes.bfloat16) \
            .reshape(NCH, C, DVS).transpose(1, 0, 2)
        s0i = np.ascontiguousarray(s0_full[:, cs]).reshape(NJ, 128, DVS)
        per_core.append(
            {**shared, "vb": np.ascontiguousarray(vbi), "s0": s0i})
    return per_core


def run(k, v, W0, trace=False, **kwargs):
    nc = _get_nc(zero_init=not np.any(np.asarray(W0)))
    in_maps = _prep_inputs(k, v, W0)
    res = run_bass_kernel_spmd(nc, in_maps, core_ids=list(range(8)),
                               trace=trace, **kwargs)
    out = np.concatenate(
        [res.results[i]["r"].astype(np.float32)     # [C, NCH, DVS]
         .transpose(1, 0, 2).reshape(T, DVS)
         for i in range(8)], axis=1
    )
    return out, res


def kernel(k, v, W0):
    out, _ = run(k, v, W0)
    return out.astype(np.float32)

